# revision 4
# baseline (speedup 1.0000x reference)
"""Trainium2 Bass kernel for SimpleRNN regressor.

Computes, for x:[B,T,F] f32:
    xp = x @ Wx + b                  # [B,T,H]
    h_t = tanh(xp_t + h_{t-1} @ Wh)  # scan over T, h0 = 0
    y = h_T @ Wd + bd                # [B,1]

Strategy (8 NeuronCores, data-parallel over batch):
  - Each core gets BC=64 batch rows. Host pre-transposes its x shard to
    [2, 128, T, BC] (f-chunk, f-in-chunk, t, b) so every DMA is a fully
    contiguous 128-partition load.
  - Per timestep, PSUM accumulates Wx_c0.T@x_c0 + Wx_c1.T@x_c1 (input
    projection, prefetchable) + Wh.T@hT (recurrent, on the critical chain),
    then one ScalarE tanh (with per-partition bias) writes hT back to SBUF.
  - State layout is transposed, hT:[H, BC], so the recurrent matmul needs
    no per-step transpose: hT_new = tanh(Wh.T @ hT + xpT_t + b).
  - 7 PSUM banks pipeline the input projections ahead of the scan chain.
"""

import numpy as np

B, T, F, H = 512, 512, 256, 64
NCORES = 8
BC = B // NCORES  # 64 batch rows per core
G = 16  # timesteps per x DMA (2 MB per transfer)

_cache = {}


def _build(t_steps=T, g=G):
    import concourse.bass as bass
    import concourse.bacc as bacc
    import concourse.mybir as mybir
    import concourse.tile as tile

    dt = mybir.dt.float32
    AF = mybir.ActivationFunctionType
    nc = bacc.Bacc("TRN2", target_bir_lowering=False, debug=False)

    xt = nc.dram_tensor("xt", [2, 128, t_steps, BC], dt, kind="ExternalInput")
    Wx = nc.dram_tensor("Wx", [F, H], dt, kind="ExternalInput")
    Wh = nc.dram_tensor("Wh", [H, H], dt, kind="ExternalInput")
    bv = nc.dram_tensor("bv", [H], dt, kind="ExternalInput")
    Wd = nc.dram_tensor("Wd", [H, 1], dt, kind="ExternalInput")
    bd = nc.dram_tensor("bd", [1], dt, kind="ExternalInput")
    y = nc.dram_tensor("y", [BC, 1], dt, kind="ExternalOutput")

    with tile.TileContext(nc) as tc:
        with (
            tc.tile_pool(name="wp", bufs=1) as wp,
            tc.tile_pool(name="xp", bufs=3) as xpool,
            tc.tile_pool(name="hp", bufs=3) as hp,
            tc.tile_pool(name="pp", bufs=7, space=bass.MemorySpace.PSUM) as pp,
            tc.tile_pool(name="fp", bufs=1, space=bass.MemorySpace.PSUM) as fp,
        ):
            # Load the tanh ACT table (~2.7us) before the scan chain needs it.
            wz = wp.tile([1, 1], dt, tag="wz")
            nc.vector.memset(wz[:], 0.0)
            wz2 = wp.tile([1, 1], dt, tag="wz2")
            nc.scalar.activation(wz2[:], wz[:], AF.Tanh)

            wx0 = wp.tile([128, H], dt, tag="wx0")
            nc.sync.dma_start(wx0[:], Wx[0:128, :])
            wx1 = wp.tile([128, H], dt, tag="wx1")
            nc.sync.dma_start(wx1[:], Wx[128:256, :])
            wh = wp.tile([H, H], dt, tag="wh")
            nc.sync.dma_start(wh[:], Wh[:, :])
            bias = wp.tile([H, 1], dt, tag="bias")
            nc.sync.dma_start(bias[:], bv[:])
            wd = wp.tile([H, 1], dt, tag="wd")
            nc.sync.dma_start(wd[:], Wd[:, :])
            bdt = wp.tile([1, 1], dt, tag="bdt")
            nc.sync.dma_start(bdt[:], bd[:])

            h_prev = None
            xa = xb = None
            for t in range(t_steps):
                grp, r = divmod(t, g)
                if r == 0:
                    xa = xpool.tile([128, g, BC], dt, tag="xa")
                    xb = xpool.tile([128, g, BC], dt, tag="xb")
                    nc.sync.dma_start(xa[:], xt[0, :, grp * g : (grp + 1) * g, :])
                    nc.sync.dma_start(xb[:], xt[1, :, grp * g : (grp + 1) * g, :])
                ps = pp.tile([H, BC], dt, tag="ps")
                nc.tensor.matmul(ps[:], wx0[:], xa[:, r, :], start=True, stop=False)
                nc.tensor.matmul(ps[:], wx1[:], xb[:, r, :], start=False, stop=(t == 0))
                if t > 0:
                    nc.tensor.matmul(ps[:], wh[:], h_prev[:], start=False, stop=True)
                h_t = hp.tile([H, BC], dt, tag="h")
                nc.scalar.activation(h_t[:], ps[:], AF.Tanh, bias=bias[:])
                h_prev = h_t

            ps2 = fp.tile([1, BC], dt, tag="ps2")
            nc.tensor.matmul(ps2[:], wd[:], h_prev[:], start=True, stop=True)
            yt = wp.tile([1, BC], dt, tag="yt")
            nc.vector.tensor_scalar_add(yt[:], ps2[:], bdt[:])
            nc.sync.dma_start(y[:, :], yt[:])

    nc.compile()
    return nc


def _prep_core_inputs(x_shard, Wx, Wh, b, Wd, bd, t_steps=T):
    bc = x_shard.shape[0]
    # [bc, t, f] -> [f, t, bc] -> [2, 128, t, bc]
    xt = np.ascontiguousarray(
        np.transpose(x_shard, (2, 1, 0)).reshape(2, 128, t_steps, bc)
    ).astype(np.float32, copy=False)
    return {
        "xt": xt,
        "Wx": np.ascontiguousarray(Wx, dtype=np.float32),
        "Wh": np.ascontiguousarray(Wh, dtype=np.float32),
        "bv": np.ascontiguousarray(b, dtype=np.float32).reshape(H),
        "Wd": np.ascontiguousarray(Wd, dtype=np.float32),
        "bd": np.ascontiguousarray(bd, dtype=np.float32).reshape(1),
    }


class _Runner:
    """Persistent PJRT executor for a prebuilt Bass module on N cores.

    Mirrors concourse.bass2jax.run_bass_via_pjrt, but keeps the jitted
    callable and device-resident inputs alive across calls so repeat
    executions skip recompilation and host->device transfer of x.
    """

    def __init__(self, nc, n_cores=NCORES):
        import jax
        import concourse.mybir as mybir
        from concourse import bass2jax
        from jax.sharding import Mesh, PartitionSpec, NamedSharding
        from jax.experimental.shard_map import shard_map

        bass2jax.install_neuronx_cc_hook()
        self.jax = jax
        self.nc = nc
        self.n_cores = n_cores

        partition_name = (
            nc.partition_id_tensor.name if nc.partition_id_tensor else None
        )
        in_names, out_names, out_avals, zero_outs = [], [], [], []
        for alloc in nc.m.functions[0].allocations:
            if not isinstance(alloc, mybir.MemoryLocationSet):
                continue
            name = alloc.memorylocations[0].name
            if alloc.kind == "ExternalInput":
                if name != partition_name:
                    in_names.append(name)
            elif alloc.kind == "ExternalOutput":
                shape = tuple(alloc.tensor_shape)
                dtype = mybir.dt.np(alloc.dtype)
                out_names.append(name)
                out_avals.append(jax.core.ShapedArray(shape, dtype))
                zero_outs.append(np.zeros(shape, dtype))
        self.in_names = in_names
        self.out_names = out_names
        self.out_avals = out_avals
        self.zero_outs = zero_outs
        n_params = len(in_names)
        n_outs = len(out_names)
        all_names = in_names + out_names
        if partition_name is not None:
            all_names = all_names + [partition_name]

        def _body(*args):
            operands = list(args)
            if partition_name is not None:
                operands.append(bass2jax.partition_id_tensor())
            outs = bass2jax._bass_exec_p.bind(
                *operands,
                out_avals=tuple(out_avals),
                in_names=tuple(all_names),
                out_names=tuple(out_names),
                lowering_input_output_aliases=(),
                sim_require_finite=True,
                sim_require_nnan=True,
                nc=nc,
            )
            return tuple(outs)

        devices = jax.devices()[:n_cores]
        assert len(devices) == n_cores, f"need {n_cores} devices"
        self.mesh = Mesh(np.asarray(devices), ("core",))
        self.sharding = NamedSharding(self.mesh, PartitionSpec("core"))
        in_specs = (PartitionSpec("core"),) * (n_params + n_outs)
        out_specs = (PartitionSpec("core"),) * n_outs
        self.donate = tuple(range(n_params, n_params + n_outs))
        self._jitted = jax.jit(
            shard_map(
                _body,
                mesh=self.mesh,
                in_specs=in_specs,
                out_specs=out_specs,
                check_rep=False,
            ),
            donate_argnums=self.donate,
            keep_unused=True,
        )
        self._dev_in = None

    def put_inputs(self, in_maps):
        concat = [
            np.concatenate([m[name] for m in in_maps], axis=0)
            for name in self.in_names
        ]
        self._dev_in = [self.jax.device_put(a, self.sharding) for a in concat]

    def run_async(self):
        zeros = [
            np.zeros((self.n_cores * z.shape[0], *z.shape[1:]), z.dtype)
            for z in self.zero_outs
        ]
        return self._jitted(*self._dev_in, *zeros)

    def run(self):
        outs = self.run_async()
        outs = [np.asarray(o) for o in outs]
        per_core = [
            {
                name: outs[i].reshape(self.n_cores, *self.out_avals[i].shape)[c]
                for i, name in enumerate(self.out_names)
            }
            for c in range(self.n_cores)
        ]
        return per_core

    def time_exec(self, iters=24, warmup=3):
        """Per-execution device time via queued-dispatch slope."""
        import time

        for _ in range(warmup):
            self.jax.block_until_ready(self.run_async())
        t0 = time.perf_counter()
        self.jax.block_until_ready(self.run_async())
        t1 = time.perf_counter()
        single = t1 - t0
        t0 = time.perf_counter()
        outs = [self.run_async() for _ in range(iters)]
        self.jax.block_until_ready(outs[-1])
        t1 = time.perf_counter()
        total = t1 - t0
        slope = (total - single) / (iters - 1)
        return {
            "single_s": single,
            "slope_s": slope,
            "total_s": total,
            "iters": iters,
        }


def _get_runner():
    if "runner" not in _cache:
        if "nc" not in _cache:
            _cache["nc"] = _build()
        _cache["runner"] = _Runner(_cache["nc"])
    return _cache["runner"]


def _run(inputs):
    x = np.asarray(inputs["x"], dtype=np.float32)
    Wx = np.asarray(inputs["Wx"], dtype=np.float32)
    Wh = np.asarray(inputs["Wh"], dtype=np.float32)
    b = np.asarray(inputs["b"], dtype=np.float32)
    Wd = np.asarray(inputs["Wd"], dtype=np.float32)
    bd = np.asarray(inputs["bd"], dtype=np.float32)

    runner = _get_runner()
    in_maps = [
        _prep_core_inputs(x[c * BC : (c + 1) * BC], Wx, Wh, b, Wd, bd)
        for c in range(NCORES)
    ]
    runner.put_inputs(in_maps)
    per_core = runner.run()
    yout = np.concatenate([r["y"] for r in per_core], axis=0)
    return yout.astype(np.float32, copy=False), runner


def kernel(**inputs):
    return _run(inputs)[0]


# revision 8
# speedup vs baseline: 2.4167x; 2.4167x over previous
"""Trainium2 Bass kernel for SimpleRNN regressor.

Computes, for x:[B,T,F] f32:
    xp = x @ Wx + b                  # [B,T,H]
    h_t = tanh(xp_t + h_{t-1} @ Wh)  # scan over T, h0 = 0
    y = h_T @ Wd + bd                # [B,1]

Strategy (8 NeuronCores, data-parallel over batch):
  - Each core gets BC=64 batch rows. Host pre-transposes its x shard to
    [2, 128, T, BC] (f-chunk, f-in-chunk, t, b) so every DMA is a fully
    contiguous 128-partition load.
  - Per timestep, PSUM accumulates Wx_c0.T@x_c0 + Wx_c1.T@x_c1 (input
    projection, prefetchable) + Wh.T@hT (recurrent, on the critical chain),
    then one ScalarE tanh (with per-partition bias) writes hT back to SBUF.
  - State layout is transposed, hT:[H, BC], so the recurrent matmul needs
    no per-step transpose: hT_new = tanh(Wh.T @ hT + xpT_t + b).
  - 7 PSUM banks pipeline the input projections ahead of the scan chain.
"""

import numpy as np

B, T, F, H = 512, 512, 256, 64
NCORES = 8
BC = B // NCORES  # 64 batch rows per core
G = 16  # timesteps per x DMA (2 MB per transfer)

_cache = {}


def _build(t_steps=T, g=G, mode="fp16", reps=1):
    import concourse.bass as bass
    import concourse.bacc as bacc
    import concourse.mybir as mybir
    import concourse.tile as tile

    dt = mybir.dt.float32
    # dth: recurrent-state/Wh/Wd dtype; dtx: x/Wx dtype (PE operand dtypes).
    # PSUM accumulation and tanh evaluation stay fp32 in all modes.
    if mode == "f32":
        dth, dtx = dt, dt
    elif mode == "bf16":
        dth, dtx = mybir.dt.bfloat16, dt
    elif mode == "fp16":
        dth, dtx = mybir.dt.float16, mybir.dt.float16
    else:
        raise ValueError(mode)
    AF = mybir.ActivationFunctionType
    nc = bacc.Bacc("TRN2", target_bir_lowering=False, debug=False)

    xt = nc.dram_tensor("xt", [2, 128, t_steps, BC], dtx, kind="ExternalInput")
    Wx = nc.dram_tensor("Wx", [F, H], dtx, kind="ExternalInput")
    Wh = nc.dram_tensor("Wh", [H, H], dth, kind="ExternalInput")
    bv = nc.dram_tensor("bv", [H], dt, kind="ExternalInput")
    Wd = nc.dram_tensor("Wd", [H, 1], dth, kind="ExternalInput")
    bd = nc.dram_tensor("bd", [1], dt, kind="ExternalInput")
    y = nc.dram_tensor("y", [BC, 1], dt, kind="ExternalOutput")

    with tile.TileContext(nc) as tc:
        with (
            tc.tile_pool(name="wp", bufs=1) as wp,
            tc.tile_pool(name="xp", bufs=3) as xpool,
            tc.tile_pool(name="hp", bufs=3) as hp,
            tc.tile_pool(name="pp", bufs=7, space=bass.MemorySpace.PSUM) as pp,
            tc.tile_pool(name="fp", bufs=1, space=bass.MemorySpace.PSUM) as fp,
        ):
            # Load the tanh ACT table (~2.7us) before the scan chain needs it.
            wz = wp.tile([1, 1], dt, tag="wz")
            nc.vector.memset(wz[:], 0.0)
            wz2 = wp.tile([1, 1], dt, tag="wz2")
            nc.scalar.activation(wz2[:], wz[:], AF.Tanh)

            wx0 = wp.tile([128, H], dtx, tag="wx0")
            nc.sync.dma_start(wx0[:], Wx[0:128, :])
            wx1 = wp.tile([128, H], dtx, tag="wx1")
            nc.sync.dma_start(wx1[:], Wx[128:256, :])
            wh = wp.tile([H, H], dth, tag="wh")
            nc.sync.dma_start(wh[:], Wh[:, :])
            bias = wp.tile([H, 1], dt, tag="bias")
            nc.sync.dma_start(bias[:], bv[:])
            wd = wp.tile([H, 1], dth, tag="wd")
            nc.sync.dma_start(wd[:], Wd[:, :])
            bdt = wp.tile([1, 1], dt, tag="bdt")
            nc.sync.dma_start(bdt[:], bd[:])

            state = {"h_prev": None}

            def body():
                xa = xb = None
                for t in range(t_steps):
                    grp, r = divmod(t, g)
                    if r == 0:
                        xa = xpool.tile([128, g, BC], dtx, tag="xa")
                        xb = xpool.tile([128, g, BC], dtx, tag="xb")
                        nc.sync.dma_start(xa[:], xt[0, :, grp * g : (grp + 1) * g, :])
                        nc.sync.dma_start(xb[:], xt[1, :, grp * g : (grp + 1) * g, :])
                    ps = pp.tile([H, BC], dt, tag="ps")
                    nc.tensor.matmul(ps[:], wx0[:], xa[:, r, :], start=True, stop=False)
                    nc.tensor.matmul(
                        ps[:], wx1[:], xb[:, r, :], start=False, stop=(t == 0)
                    )
                    if t > 0:
                        nc.tensor.matmul(
                            ps[:], wh[:], state["h_prev"][:], start=False, stop=True
                        )
                    h_t = hp.tile([H, BC], dth, tag="h")
                    nc.scalar.activation(h_t[:], ps[:], AF.Tanh, bias=bias[:])
                    state["h_prev"] = h_t

            if reps == 1:
                body()
            else:
                with tc.For_i(0, reps, 1):
                    body()
            h_prev = state["h_prev"]

            ps2 = fp.tile([1, BC], dt, tag="ps2")
            nc.tensor.matmul(ps2[:], wd[:], h_prev[:], start=True, stop=True)
            yt = wp.tile([1, BC], dt, tag="yt")
            nc.vector.tensor_scalar_add(yt[:], ps2[:], bdt[:])
            nc.sync.dma_start(y[:, :], yt[:])

    nc.compile()
    return nc


def _prep_core_inputs(x_shard, Wx, Wh, b, Wd, bd, t_steps=T, mode="fp16"):
    if mode == "f32":
        dth, dtx = np.float32, np.float32
    elif mode == "bf16":
        import ml_dtypes

        dth, dtx = ml_dtypes.bfloat16, np.float32
    elif mode == "fp16":
        dth, dtx = np.float16, np.float16
    else:
        raise ValueError(mode)
    bc = x_shard.shape[0]
    # [bc, t, f] -> [f, t, bc] -> [2, 128, t, bc]
    xt = np.ascontiguousarray(
        np.transpose(x_shard, (2, 1, 0)).reshape(2, 128, t_steps, bc)
    ).astype(dtx)
    return {
        "xt": xt,
        "Wx": np.ascontiguousarray(Wx).astype(dtx),
        "Wh": np.ascontiguousarray(Wh).astype(dth),
        "bv": np.ascontiguousarray(b, dtype=np.float32).reshape(H),
        "Wd": np.ascontiguousarray(Wd).astype(dth),
        "bd": np.ascontiguousarray(bd, dtype=np.float32).reshape(1),
    }


class _Runner:
    """Persistent PJRT executor for a prebuilt Bass module on N cores.

    Mirrors concourse.bass2jax.run_bass_via_pjrt, but keeps the jitted
    callable and device-resident inputs alive across calls so repeat
    executions skip recompilation and host->device transfer of x.
    """

    def __init__(self, nc, n_cores=NCORES):
        import jax
        import concourse.mybir as mybir
        from concourse import bass2jax
        from jax.sharding import Mesh, PartitionSpec, NamedSharding
        from jax.experimental.shard_map import shard_map

        bass2jax.install_neuronx_cc_hook()
        self.jax = jax
        self.nc = nc
        self.n_cores = n_cores

        partition_name = (
            nc.partition_id_tensor.name if nc.partition_id_tensor else None
        )
        in_names, out_names, out_avals, zero_outs = [], [], [], []
        for alloc in nc.m.functions[0].allocations:
            if not isinstance(alloc, mybir.MemoryLocationSet):
                continue
            name = alloc.memorylocations[0].name
            if alloc.kind == "ExternalInput":
                if name != partition_name:
                    in_names.append(name)
            elif alloc.kind == "ExternalOutput":
                shape = tuple(alloc.tensor_shape)
                dtype = mybir.dt.np(alloc.dtype)
                out_names.append(name)
                out_avals.append(jax.core.ShapedArray(shape, dtype))
                zero_outs.append(np.zeros(shape, dtype))
        self.in_names = in_names
        self.out_names = out_names
        self.out_avals = out_avals
        self.zero_outs = zero_outs
        n_params = len(in_names)
        n_outs = len(out_names)
        all_names = in_names + out_names
        if partition_name is not None:
            all_names = all_names + [partition_name]

        def _body(*args):
            operands = list(args)
            if partition_name is not None:
                operands.append(bass2jax.partition_id_tensor())
            outs = bass2jax._bass_exec_p.bind(
                *operands,
                out_avals=tuple(out_avals),
                in_names=tuple(all_names),
                out_names=tuple(out_names),
                lowering_input_output_aliases=(),
                sim_require_finite=True,
                sim_require_nnan=True,
                nc=nc,
            )
            return tuple(outs)

        devices = jax.devices()[:n_cores]
        assert len(devices) == n_cores, f"need {n_cores} devices"
        self.mesh = Mesh(np.asarray(devices), ("core",))
        self.sharding = NamedSharding(self.mesh, PartitionSpec("core"))
        in_specs = (PartitionSpec("core"),) * (n_params + n_outs)
        out_specs = (PartitionSpec("core"),) * n_outs
        self.donate = tuple(range(n_params, n_params + n_outs))
        self._jitted = jax.jit(
            shard_map(
                _body,
                mesh=self.mesh,
                in_specs=in_specs,
                out_specs=out_specs,
                check_rep=False,
            ),
            donate_argnums=self.donate,
            keep_unused=True,
        )
        self._dev_in = None

    def put_inputs(self, in_maps):
        concat = [
            np.concatenate([m[name] for m in in_maps], axis=0)
            for name in self.in_names
        ]
        self._dev_in = [self.jax.device_put(a, self.sharding) for a in concat]

    def run_async(self):
        zeros = [
            np.zeros((self.n_cores * z.shape[0], *z.shape[1:]), z.dtype)
            for z in self.zero_outs
        ]
        return self._jitted(*self._dev_in, *zeros)

    def run(self):
        outs = self.run_async()
        outs = [np.asarray(o) for o in outs]
        per_core = [
            {
                name: outs[i].reshape(self.n_cores, *self.out_avals[i].shape)[c]
                for i, name in enumerate(self.out_names)
            }
            for c in range(self.n_cores)
        ]
        return per_core

    def time_exec(self, iters=24, warmup=3):
        """Per-execution device time via queued-dispatch slope."""
        import time

        for _ in range(warmup):
            self.jax.block_until_ready(self.run_async())
        t0 = time.perf_counter()
        self.jax.block_until_ready(self.run_async())
        t1 = time.perf_counter()
        single = t1 - t0
        t0 = time.perf_counter()
        outs = [self.run_async() for _ in range(iters)]
        self.jax.block_until_ready(outs[-1])
        t1 = time.perf_counter()
        total = t1 - t0
        slope = (total - single) / (iters - 1)
        return {
            "single_s": single,
            "slope_s": slope,
            "total_s": total,
            "iters": iters,
        }


def _get_runner():
    if "runner" not in _cache:
        if "nc" not in _cache:
            _cache["nc"] = _build()
        _cache["runner"] = _Runner(_cache["nc"])
    return _cache["runner"]


def _run(inputs):
    x = np.asarray(inputs["x"], dtype=np.float32)
    Wx = np.asarray(inputs["Wx"], dtype=np.float32)
    Wh = np.asarray(inputs["Wh"], dtype=np.float32)
    b = np.asarray(inputs["b"], dtype=np.float32)
    Wd = np.asarray(inputs["Wd"], dtype=np.float32)
    bd = np.asarray(inputs["bd"], dtype=np.float32)

    runner = _get_runner()
    in_maps = [
        _prep_core_inputs(x[c * BC : (c + 1) * BC], Wx, Wh, b, Wd, bd)
        for c in range(NCORES)
    ]
    runner.put_inputs(in_maps)
    per_core = runner.run()
    yout = np.concatenate([r["y"] for r in per_core], axis=0)
    return yout.astype(np.float32, copy=False), runner


def kernel(**inputs):
    return _run(inputs)[0]
